# revision 1
# baseline (speedup 1.0000x reference)
"""CGCNN TRN2 kernel: 8-core SPMD edge pipeline + single-core pooling/classifier."""
import numpy as np

G = 16
SLOTS = 2


def preprocess(x, edge_index, edge_attr, batch, params, n_dev=8, ncol=512):
    N = x.shape[0]
    npd = N // n_dev
    ntiles = (npd + 127) // 128
    SB = G * ncol
    spsb = SLOTS * ncol          # slots per superblock (1024)
    src = np.asarray(edge_index[0], dtype=np.int64)
    dst = np.asarray(edge_index[1], dtype=np.int64)
    attr = np.asarray(edge_attr, dtype=np.float32)

    # per (dev, tile) slot counts
    devdata = []
    D8 = 0
    for d in range(n_dev):
        lo = d * npd
        sel = np.where((dst >= lo) & (dst < lo + npd))[0]
        order = np.argsort(dst[sel], kind="stable")
        eid = sel[order]
        ed = (dst[eid] - lo).astype(np.int64)
        cnt = np.bincount(ed, minlength=npd)
        scnt = (cnt + 7) // 8                       # 8-slots per node
        tile_of_node = np.arange(npd) // 128
        tslots = np.bincount(tile_of_node, weights=scnt, minlength=ntiles).astype(np.int64)
        D8 = max(D8, int(tslots.max()))
        devdata.append((lo, eid, ed, cnt, scnt))
    D8 = int(D8)

    nslots = ntiles * D8
    nsb = (nslots + spsb - 1) // spsb
    out = []
    for d in range(n_dev):
        lo, eid, ed, cnt, scnt = devdata[d]
        # slot index of each node's first slot
        node_slot0 = np.zeros(npd, np.int64)
        for T in range(ntiles):
            nlo = T * 128
            nhi = min(nlo + 128, npd)
            c = np.concatenate(([0], np.cumsum(scnt[nlo:nhi])[:-1]))
            node_slot0[nlo:nhi] = T * D8 + c
        # per-edge padded position: q = slot*8 + rank%...
        estart = np.concatenate(([0], np.cumsum(cnt)[:-1]))  # first sorted-edge per node
        rank = np.arange(len(ed)) - estart[ed]
        slot = node_slot0[ed] + rank // 8
        qpos = slot * 8 + rank % 8          # position in padded stream *within slot*8
        # map slot -> (sb, n, s); edge q -> (sb, n, g)
        # slot iota: sb = iota // spsb ; rem = iota % spsb ; n = rem // 2 ; s = rem % 2
        # edge stream pos = sb*SB + n*G + 8s + r
        si = slot
        sb_e = si // spsb
        rem = si % spsb
        n_e = rem // SLOTS
        s_e = rem % SLOTS
        g_e = 8 * s_e + (rank % 8)
        t_e = n_e // 128
        p_e = n_e % 128

        gidx = np.zeros((nsb, 128, 4 * 18), np.int32)
        gidx[sb_e, p_e, t_e * 18 + g_e] = src[eid]
        attr_sw = np.zeros((nsb, 80, ncol), np.float32)
        for c in range(4):
            attr_sw[sb_e, g_e * 5 + c, n_e] = attr[eid, c]
        rho = np.ones((nsb, 80, ncol), np.float32)
        # rho row occupies 5g+4; mark real edges 0
        attr_sw[sb_e, g_e * 5 + 4, n_e] = 0.0
        pad_rho = np.ones((nsb, G, ncol), np.float32)
        pad_rho[sb_e, g_e, n_e] = 0.0
        for gg in range(G):
            attr_sw[:, gg * 5 + 4, :] = pad_rho[:, gg, :]

        # per-slot node (for xd gather and mrel): -1 for pad slots
        slot_node = np.full(nsb * spsb, -1, np.int64)
        iot = np.arange(nslots)
        Ts = iot // D8
        # valid slots of tile T: first sum(scnt in tile) slots
        used = np.zeros(nslots, bool)
        nd_of = np.full(nslots, -1, np.int64)
        for T in range(ntiles):
            nlo = T * 128
            nhi = min(nlo + 128, npd)
            k = int(scnt[nlo:nhi].sum())
            nd_of[T * D8:T * D8 + k] = np.repeat(np.arange(nlo, nhi), scnt[nlo:nhi])
            used[T * D8:T * D8 + k] = True
        slot_node[:nslots] = nd_of
        sn = slot_node.reshape(nsb, ncol, SLOTS)   # [sb, n, s]

        # xd gather index (global node id; 0 for pads)
        for s in range(SLOTS):
            v = sn[:, :, s]                        # [nsb, ncol]
            t_all = (np.arange(ncol) // 128)[None, :].repeat(nsb, 0)
            p_all = (np.arange(ncol) % 128)[None, :].repeat(nsb, 0)
            gidx[np.arange(nsb)[:, None], p_all, t_all * 18 + 16 + s] = \
                np.maximum(v + lo, 0).astype(np.int32)

        # mrel: [nsb, 128, 16] int8: col 2*(2c+s)+part: rel for partials in
        # that part's tile, -1 elsewhere
        mrelv = np.full((nsb, 128, 16), -1, np.int8)
        for c in range(4):
            for s in range(SLOTS):
                ncols = np.arange(128 * c, 128 * (c + 1))
                v = sn[:, ncols, s]                # [nsb, 128]
                iota_slot = (np.arange(nsb) * spsb)[:, None] + ncols[None, :] * SLOTS + s
                Tt = iota_slot // D8
                rel = v - 128 * Tt
                rel[v < 0] = -1
                assert ((rel >= -1) & (rel < 128)).all()
                T0 = Tt[:, 0:1]
                for part in range(2):
                    rp = np.where((Tt == T0 + part) & (rel >= 0), rel, -1)
                    mrelv[:, :, 2 * (2 * c + s) + part] = rp.astype(np.int8)

        xsl = np.zeros((ntiles * 128, 3), np.float32)
        xsl[:npd] = np.asarray(x[lo:lo + npd], np.float32)
        out.append(dict(gidx=gidx, attr_sw=attr_sw, mrel=mrelv, xsl=xsl, lo=lo))

    # uniform merge split schedule: per (sb, c, s): partition threshold per tile
    # slots iota(p) = sb*spsb + (128c+p)*2 + s ; tile = iota // D8
    msched = []
    for b in range(nsb):
        for c in range(4):
            for s in range(SLOTS):
                io0 = b * spsb + (128 * c) * 2 + s
                iolast = io0 + 127 * 2
                T0 = io0 // D8
                T1 = iolast // D8
                col0 = 2 * (2 * c + s)
                if T0 == T1:
                    msched.append((b, c, s, [(T0, col0)]))
                else:
                    assert (io0 + 2 * 127) // D8 == T0 + 1, "D8 too small"
                    msched.append((b, c, s, [(T0, col0), (T0 + 1, col0 + 1)]))

    WT = {}
    for l in (1, 2, 3):
        Wf, bf = params[f"Wf{l}"], params[f"bf{l}"]
        Ws_, bs = params[f"Ws{l}"], params[f"bs{l}"]
        Wx = np.zeros((3 * G, 112), np.float32)
        Wd6 = np.zeros((3 * SLOTS, 112), np.float32)
        Wa = np.zeros((5 * G, 112), np.float32)
        for gg in range(G):
            ss = gg // 8
            for j in range(3):
                mg = 3 * gg + j
                mc = 64 + 3 * gg + j
                for f in range(3):
                    Wx[3 * gg + f, mg] = Wf[j, 3 + f]
                    Wx[3 * gg + f, mc] = Ws_[j, 3 + f]
                    Wd6[3 * ss + f, mg] = Wf[j, f]
                    Wd6[3 * ss + f, mc] = Ws_[j, f]
                for c in range(4):
                    Wa[5 * gg + c, mg] = Wf[j, 6 + c]
                    Wa[5 * gg + c, mc] = Ws_[j, 6 + c]
                Wa[5 * gg + 4, mc] = -30000.0
        bias_g = np.zeros((48, 1), np.float32)
        bias_c = np.zeros((48, 1), np.float32)
        for gg in range(G):
            for j in range(3):
                bias_g[3 * gg + j, 0] = bf[j]
                bias_c[3 * gg + j, 0] = bs[j]
        Wxd = np.zeros((70, 112), np.float32)
        Wxd[0:48] = Wx
        Wxd[64:70] = Wd6
        WT[l] = dict(Wx=Wx, Wd6=Wd6, Wxd=Wxd, Wa=Wa, bias_g=bias_g,
                     bias_c=bias_c, bias_cm30=bias_c - 30.0)

    S = np.zeros((48, 6), np.float32)
    for ss in range(SLOTS):
        for rr in range(8):
            for f in range(3):
                S[24 * ss + 3 * rr + f, 3 * ss + f] = 1.0
    iota = np.tile(np.arange(128, dtype=np.float32), (128, 1))

    shared = dict(WT=WT, S=S, iota=iota, msched=msched, nsb=nsb, ncol=ncol,
                  npd=npd, ntiles=ntiles, D8=D8, n_dev=n_dev)
    return out, shared

import numpy as np
import concourse.bass as bass
import concourse.bacc as bacc
import concourse.tile as tile
import concourse.mybir as mybir
from concourse.masks import make_identity

F32 = mybir.dt.float32
AF = mybir.ActivationFunctionType
OP = mybir.AluOpType
G = 16
SLOTS = 2


def build_spmd(shared, n_dev=8, N=100000):
    nsb, ncol, npd, ntiles = (shared["nsb"], shared["ncol"], shared["npd"],
                              shared["ntiles"])
    msched = shared["msched"]
    nc = bacc.Bacc("TRN2", target_bir_lowering=False, debug=False,
                   num_devices=n_dev)

    x_in = nc.dram_tensor("x", [N, 3], F32, kind="ExternalInput").ap()
    xsl_in = nc.dram_tensor("xsl", [ntiles * 128, 3], F32, kind="ExternalInput").ap()
    gidx = nc.dram_tensor("gidx", [nsb, 128, 72], mybir.dt.int32,
                          kind="ExternalInput").ap()
    attr_sw = nc.dram_tensor("attr_sw", [nsb, 80, ncol], F32,
                             kind="ExternalInput").ap()
    mrel = nc.dram_tensor("mrel", [nsb, 128, 16], mybir.dt.int8,
                          kind="ExternalInput").ap()
    wts = {}
    for l in (1, 2, 3):
        for nm, shp in (("Wxd", [70, 112]), ("Wa", [80, 112]),
                        ("bias_g", [48, 1]), ("bias_c", [48, 1]),
                        ("bias_cm30", [48, 1])):
            wts[(l, nm)] = nc.dram_tensor(f"{nm}{l}", shp, F32,
                                          kind="ExternalInput").ap()
    S_in = nc.dram_tensor("S", [48, 6], F32, kind="ExternalInput").ap()
    iota_in = nc.dram_tensor("iota", [128, 128], F32, kind="ExternalInput").ap()
    xouts = [nc.dram_tensor(f"xo{l}", [ntiles * 128, 3], F32,
                            kind="ExternalOutput").ap() for l in (1, 2, 3)]

    from contextlib import ExitStack
    with tile.TileContext(nc) as tc, ExitStack() as _es:
        cp = _es.enter_context(tc.tile_pool(name="const", bufs=1))
        ident = cp.tile([128, 128], F32)
        make_identity(nc, ident[:])
        iota_t = cp.tile([128, 128], F32)
        nc.sync.dma_start(out=iota_t[:], in_=iota_in[:])
        S_t = cp.tile([48, 6], F32)
        nc.sync.dma_start(out=S_t[:], in_=S_in[:])
        wt = {}
        for (l, nm), ap_ in wts.items():
            t = cp.tile(list(ap_.shape), F32, name=f"w{nm}{l}")
            nc.sync.dma_start(out=t[:], in_=ap_[:])
            wt[(l, nm)] = t
        x_sb = cp.tile([128, ntiles * 3], F32)
        nc.sync.dma_start(
            out=x_sb[:].rearrange("p (T f) -> p T f", f=3),
            in_=xsl_in[:].rearrange("(T p) f -> p T f", p=128))
        agg_sb = cp.tile([128, ntiles * 3], F32)

        dp = _es.enter_context(tc.tile_pool(name="dram", bufs=1, space="DRAM"))
        xfulls = [dp.tile([N, 3], F32, name=f"xfull{l}") for l in (1, 2)]
        xslo = [dp.tile([ntiles * 128, 3], F32, name=f"xslo{l}") for l in (1, 2)]

        sb = _es.enter_context(tc.tile_pool(name="work", bufs=6))
        ps = _es.enter_context(tc.tile_pool(name="psum", bufs=2, space="PSUM"))

        for l in (1, 2, 3):
            xsrc = x_in if l == 1 else xfulls[l - 2][:]
            nc.vector.memset(agg_sb[:], 0.0)
            si = 0
            for b in range(nsb):
                git = sb.tile([128, 72], mybir.dt.int32, tag="git")
                nc.sync.dma_start(out=git[:], in_=gidx[b])
                mrt = sb.tile([128, 16], mybir.dt.int8, tag="mrt")
                nc.sync.dma_start(out=mrt[:], in_=mrel[b])
                mrf = sb.tile([128, 16], F32, tag="mrf")
                nc.vector.tensor_copy(out=mrf[:], in_=mrt[:])
                at = sb.tile([80, ncol], F32, tag="at")
                nc.sync.dma_start(out=at[:], in_=attr_sw[b])
                Z = sb.tile([70, ncol], F32, tag="Z")
                for t in range(4):
                    preT = sb.tile([128, 70], F32, tag="preT")
                    for g in range(G):
                        nc.gpsimd.indirect_dma_start(
                            out=preT[:, 3 * g:3 * g + 3], out_offset=None,
                            in_=xsrc,
                            in_offset=bass.IndirectOffsetOnAxis(
                                ap=git[:, 18 * t + g:18 * t + g + 1], axis=0))
                    for s in range(SLOTS):
                        nc.gpsimd.indirect_dma_start(
                            out=preT[:, 64 + 3 * s:67 + 3 * s], out_offset=None,
                            in_=xsrc,
                            in_offset=bass.IndirectOffsetOnAxis(
                                ap=git[:, 18 * t + 16 + s:18 * t + 17 + s],
                                axis=0))
                    tp = ps.tile([70, 128], F32, tag="tp")
                    nc.tensor.transpose(out=tp[:], in_=preT[:], identity=ident[:])
                    nc.scalar.copy(out=Z[:, 128 * t:128 * (t + 1)], in_=tp[:])
                L = ps.tile([112, ncol], F32, tag="L")
                nc.tensor.matmul(out=L[:], lhsT=wt[(l, "Wxd")][0:48, :],
                                 rhs=Z[0:48, :], start=True, stop=False)
                nc.tensor.matmul(out=L[:], lhsT=wt[(l, "Wxd")][64:70, :],
                                 rhs=Z[64:70, :], start=False, stop=False)
                nc.tensor.matmul(out=L[:], lhsT=wt[(l, "Wa")][:], rhs=at[:],
                                 start=False, stop=True)
                gate = sb.tile([48, ncol], F32, tag="gate")
                nc.scalar.activation(out=gate[:], in_=L[0:48, :], func=AF.Sigmoid,
                                     bias=wt[(l, "bias_g")][:, 0:1])
                mcl = sb.tile([48, ncol], F32, tag="mcl")
                nc.vector.tensor_scalar(
                    out=mcl[:], in0=L[64:112, :],
                    scalar1=wt[(l, "bias_c")][:, 0:1], scalar2=30.0,
                    op0=OP.add, op1=OP.min)
                rl = sb.tile([48, ncol], F32, tag="rl")
                nc.scalar.activation(out=rl[:], in_=L[64:112, :], func=AF.Relu,
                                     bias=wt[(l, "bias_cm30")][:, 0:1])
                ex = sb.tile([48, ncol], F32, tag="ex")
                nc.scalar.activation(out=ex[:], in_=mcl[:], func=AF.Exp)
                corev = sb.tile([48, ncol], F32, tag="corev")
                nc.scalar.activation(out=corev[:], in_=ex[:], func=AF.Ln,
                                     bias=1.0)
                nc.vector.tensor_tensor(out=corev[:], in0=corev[:], in1=rl[:],
                                        op=OP.add)
                msg = sb.tile([48, ncol], F32, tag="msg")
                nc.vector.tensor_tensor(out=msg[:], in0=gate[:], in1=corev[:],
                                        op=OP.mult)
                p8 = ps.tile([6, ncol], F32, tag="p8")
                nc.tensor.matmul(out=p8[:], lhsT=S_t[:], rhs=msg[:],
                                 start=True, stop=True)
                p8s = sb.tile([6, ncol], F32, tag="p8s")
                nc.scalar.copy(out=p8s[:], in_=p8[:])
                for c in range(4):
                    tp2 = ps.tile([128, 6], F32, tag="tp")
                    nc.tensor.transpose(out=tp2[:],
                                        in_=p8s[:, 128 * c:128 * (c + 1)],
                                        identity=ident[0:6, 0:6])
                    tps = sb.tile([128, 6], F32, tag="tps")
                    nc.scalar.copy(out=tps[:], in_=tp2[:])
                    for s in range(SLOTS):
                        eb, ec, es, parts = msched[si]
                        si += 1
                        for (T, colidx) in parts:
                            if T >= ntiles:
                                continue
                            Sel = sb.tile([128, 128], F32, tag="Sel")
                            nc.vector.tensor_tensor(
                                out=Sel[:],
                                in0=mrf[:, colidx:colidx + 1].to_broadcast(
                                    [128, 128]),
                                in1=iota_t[:], op=OP.is_equal)
                            selp = ps.tile([128, 3], F32, tag="selp")
                            nc.tensor.matmul(out=selp[:], lhsT=Sel[:],
                                             rhs=tps[:, 3 * s:3 * s + 3],
                                             start=True, stop=True)
                            nc.vector.tensor_tensor(
                                out=agg_sb[:, 3 * T:3 * T + 3],
                                in0=agg_sb[:, 3 * T:3 * T + 3],
                                in1=selp[:], op=OP.add)
            nc.vector.tensor_tensor(out=x_sb[:], in0=x_sb[:], in1=agg_sb[:],
                                    op=OP.add)
            nc.sync.dma_start(
                out=xouts[l - 1][:].rearrange("(T p) f -> p T f", p=128),
                in_=x_sb[:].rearrange("p (T f) -> p T f", f=3))
            if l < 3:
                nc.sync.dma_start(
                    out=xslo[l - 1][:].rearrange("(T p) f -> p T f", p=128),
                    in_=x_sb[:].rearrange("p (T f) -> p T f", f=3))
                nc.gpsimd.collective_compute(
                    "AllGather", OP.bypass,
                    replica_groups=[list(range(n_dev))],
                    ins=[xslo[l - 1][0:npd, :].opt()],
                    outs=[xfulls[l - 1][:].opt()])
    nc.compile()
    return nc


def build_final(batch, N=100000, NG=64):
    """Single-core pooling + classifier. batch: host numpy array (sorted)."""
    nc = bacc.Bacc("TRN2", target_bir_lowering=False, debug=False, num_devices=1)
    xTs = [nc.dram_tensor(f"x{l}T", [3, N], F32, kind="ExternalInput").ap()
           for l in (1, 2, 3)]
    WlT_in = nc.dram_tensor("WlT", [3, 128], F32, kind="ExternalInput").ap()
    WclsT_in = nc.dram_tensor("WclsT", [128, 144], F32, kind="ExternalInput").ap()
    bcls_in = nc.dram_tensor("bcls", [64, 144], F32, kind="ExternalInput").ap()
    out_t = nc.dram_tensor("out", [NG, 144], F32, kind="ExternalOutput").ap()

    bnds = [0]
    for grp in range(NG):
        bnds.append(int(np.searchsorted(batch, grp, side="right")))

    from contextlib import ExitStack
    with tile.TileContext(nc) as tc, ExitStack() as _es:
        cp = _es.enter_context(tc.tile_pool(name="const", bufs=1))
        WlT_t = cp.tile([3, 128], F32)
        nc.sync.dma_start(out=WlT_t[:], in_=WlT_in[:])
        WclsT_t = cp.tile([128, 144], F32)
        nc.sync.dma_start(out=WclsT_t[:], in_=WclsT_in[:])
        bcls_t = cp.tile([64, 144], F32)
        nc.sync.dma_start(out=bcls_t[:], in_=bcls_in[:])
        PSsum = cp.tile([128, NG], F32)
        nc.vector.memset(PSsum[:], 0.0)
        sb = _es.enter_context(tc.tile_pool(name="work", bufs=6))
        ps = _es.enter_context(tc.tile_pool(name="psum", bufs=2, space="PSUM"))
        CH = 2048
        for l in (1, 2, 3):
            Pl = sb.tile([128, NG], F32, tag="Pl")
            nc.vector.memset(Pl[:], -3.0e38)
            for c0 in range(0, N, CH):
                cw = min(CH, N - c0)
                xt = sb.tile([3, CH], F32, tag="xt")
                nc.sync.dma_start(out=xt[:, 0:cw], in_=xTs[l - 1][:, c0:c0 + cw])
                for q0 in range(0, cw, 512):
                    qw = min(512, cw - q0)
                    yp = ps.tile([128, 512], F32, tag="yp")
                    nc.tensor.matmul(out=yp[:, 0:qw], lhsT=WlT_t[:],
                                     rhs=xt[:, q0:q0 + qw], start=True, stop=True)
                    a = c0 + q0
                    bz = a + qw
                    for grp in range(NG):
                        g0 = max(bnds[grp], a)
                        g1 = min(bnds[grp + 1], bz)
                        if g0 >= g1:
                            continue
                        tmp = sb.tile([128, 1], F32, tag="tmp")
                        nc.vector.tensor_reduce(
                            out=tmp[:], in_=yp[:, g0 - a:g1 - a],
                            axis=mybir.AxisListType.X, op=OP.max)
                        nc.vector.tensor_tensor(out=Pl[:, grp:grp + 1],
                                                in0=Pl[:, grp:grp + 1],
                                                in1=tmp[:], op=OP.max)
            nc.vector.tensor_tensor(out=PSsum[:], in0=PSsum[:], in1=Pl[:],
                                    op=OP.add)
        op_ = ps.tile([64, 144], F32, tag="yp")
        nc.tensor.matmul(out=op_[:], lhsT=PSsum[:, 0:64], rhs=WclsT_t[:],
                         start=True, stop=True)
        ot = sb.tile([64, 144], F32, tag="ot")
        nc.vector.tensor_tensor(out=ot[:], in0=op_[:], in1=bcls_t[:], op=OP.add)
        nc.sync.dma_start(out=out_t[:], in_=ot[:])
    nc.compile()
    return nc

import numpy as np

_CACHE = {}


def kernel(**inputs):
    from concourse import bass_utils
    x = np.asarray(inputs["x"], np.float32)
    ei = np.asarray(inputs["edge_index"])
    ea = np.asarray(inputs["edge_attr"], np.float32)
    batch = np.asarray(inputs["batch"])
    n_dev = 8
    N = x.shape[0]

    devs, shared = preprocess(x, ei, ea, batch, inputs, n_dev=n_dev)
    npd, ntiles, nsb = shared["npd"], shared["ntiles"], shared["nsb"]

    key = ("spmd", nsb, shared["D8"], ntiles)
    if key not in _CACHE:
        _CACHE[key] = build_spmd(shared, n_dev=n_dev, N=N)
    nc1 = _CACHE[key]

    WT = shared["WT"]
    in_maps = []
    for d in range(n_dev):
        dv = devs[d]
        m = dict(x=x, xsl=dv["xsl"], gidx=dv["gidx"], attr_sw=dv["attr_sw"],
                 mrel=dv["mrel"], S=shared["S"],
                 iota=shared["iota"].astype(np.float32))
        for l in (1, 2, 3):
            m[f"Wxd{l}"] = WT[l]["Wxd"]
            m[f"Wa{l}"] = WT[l]["Wa"]
            m[f"bias_g{l}"] = WT[l]["bias_g"]
            m[f"bias_c{l}"] = WT[l]["bias_c"]
            m[f"bias_cm30{l}"] = WT[l]["bias_cm30"]
        in_maps.append(m)
    r1 = bass_utils.run_bass_kernel_spmd(nc1, in_maps, core_ids=list(range(n_dev)))

    xTs = {}
    for li, l in enumerate((1, 2, 3)):
        full = np.concatenate(
            [r1.results[d][f"xo{l}"][:npd] for d in range(n_dev)], axis=0)
        xTs[l] = np.ascontiguousarray(full.T)

    key2 = ("final", N)
    if key2 not in _CACHE:
        _CACHE[key2] = build_final(np.asarray(batch, np.int64), N=N)
    nc2 = _CACHE[key2]

    W_cls = np.asarray(inputs["W_cls"], np.float32)
    b_eff = (np.asarray(inputs["b_cls"], np.float32)
             + 3.0 * W_cls @ np.asarray(inputs["b_lin"], np.float32))
    fin = dict(x1T=xTs[1], x2T=xTs[2], x3T=xTs[3],
               WlT=np.ascontiguousarray(np.asarray(inputs["W_lin"], np.float32).T),
               WclsT=np.ascontiguousarray(W_cls.T),
               bcls=np.tile(b_eff[None, :], (64, 1)))
    r2 = bass_utils.run_bass_kernel_spmd(nc2, [fin], core_ids=[0])
    return r2.results[0]["out"].astype(np.float32)

